# revision 16
# baseline (speedup 1.0000x reference)
"""Trainium2 Bass kernel for nn_MultiHeadMaskedAttention (B=2, S=2048, D=1024, H=16).

Sharding: 8-way tensor-parallel over heads (2 heads/core, both batches on every
core). Each core projects Q^T/K^T/V for its heads, runs causal attention in
transposed-score layout (S^T blocks = K^T-band.T @ Q^T-band, exp without
max-subtraction, denominator via a ones-column appended to V), normalizes, then
one 8-core AllToAll routes o^T columns so core c ends up with the full
1024-channel o^T for rows [512c, 512c+512) of concat(batch0, batch1). Each core
then does the output projection for its rows, adds bias + residual (values),
LayerNorms, and returns its [512, 1024] slice.

Matmul dtype is float32r (tf32-class, full PE rate at N=512) by default; the
exchange + output projection stage runs in bf16. Emission order ladders
projection s-halves with attention q-chunks so DMA streams under PE compute.
"""

import numpy as np

import concourse.bass as bass
import concourse.mybir as mybir
import concourse.tile as tile
from concourse import bacc

B, S, D, H = 2, 2048, 1024, 16
HD = D // H            # 64
N_CORES = 8
HPC = H // N_CORES     # 2 heads per core
CPC = HPC * HD         # 128 channels per core
SCALE = HD ** -0.5
EPS = 1e-12
QCH = 512              # q-chunk (free dim of S^T blocks)
NQ = S // QCH          # 4
NSB = S // 128         # 16 s-chunks
KCH = D // 128         # 8 contraction chunks

F32 = mybir.dt.float32
AF = mybir.ActivationFunctionType
ALU = mybir.AluOpType

_CACHE = {}


def _build(causal: bool, dt_name: str, xdt_name: str):
    DT = getattr(mybir.dt, dt_name)
    XDT = getattr(mybir.dt, xdt_name)
    nc = bacc.Bacc("TRN2", target_bir_lowering=False, debug=False,
                   num_devices=N_CORES)

    # ---- I/O ----
    xqT = nc.dram_tensor("xqT", [B, D, S], DT, kind="ExternalInput")
    xkT = nc.dram_tensor("xkT", [B, D, S], DT, kind="ExternalInput")
    xvT = nc.dram_tensor("xvT", [B, D, S], DT, kind="ExternalInput")
    wqT = nc.dram_tensor("wqT", [D, CPC], DT, kind="ExternalInput")
    wkT = nc.dram_tensor("wkT", [D, CPC], DT, kind="ExternalInput")
    wvT = nc.dram_tensor("wvT", [D, CPC], DT, kind="ExternalInput")
    woT = nc.dram_tensor("woT", [D, D], XDT, kind="ExternalInput")
    ident_in = nc.dram_tensor("ident_in", [128, 128], DT, kind="ExternalInput")
    bq_in = nc.dram_tensor("bq_in", [CPC, 1], F32, kind="ExternalInput")
    bk_in = nc.dram_tensor("bk_in", [CPC, 1], F32, kind="ExternalInput")
    bv_in = nc.dram_tensor("bv_in", [CPC, 1], F32, kind="ExternalInput")
    bo_in = nc.dram_tensor("bo_in", [1, D], F32, kind="ExternalInput")
    gamma_in = nc.dram_tensor("gamma_in", [1, D], F32, kind="ExternalInput")
    beta_in = nc.dram_tensor("beta_in", [1, D], F32, kind="ExternalInput")
    res_in = nc.dram_tensor("res_in", [QCH, D], F32, kind="ExternalInput")
    if causal:
        masks_in = nc.dram_tensor("masks_in", [128, 2, 4, QCH], DT, kind="ExternalInput")
    else:
        maskfull_in = nc.dram_tensor("maskfull_in", [NSB, 128, NQ, QCH], F32,
                                     kind="ExternalInput")
    out = nc.dram_tensor("out", [QCH, D], F32, kind="ExternalOutput")

    with tile.TileContext(nc) as tc:
        with (
            tc.tile_pool(name="const", bufs=1) as const,
            tc.tile_pool(name="xs", bufs=4) as xs,
            tc.tile_pool(name="qk", bufs=1) as qk,
            tc.tile_pool(name="vt", bufs=2) as vtp,
            tc.tile_pool(name="pts", bufs=6) as pts,
            tc.tile_pool(name="sm", bufs=4) as sm,
            tc.tile_pool(name="epi", bufs=2) as epi,
            tc.tile_pool(name="ps2", bufs=4, space="PSUM") as ps2,
            tc.tile_pool(name="dram", bufs=1, space="DRAM") as dram,
        ):
            # ---- early constants (gpsimd DMA queue; xt streams use sync) ----
            wq_sb = const.tile([128, KCH, CPC], DT)
            nc.sync.dma_start(out=wq_sb, in_=wqT.rearrange("(a p) c -> p a c", p=128))
            bq_sb = const.tile([CPC, 1], F32)
            nc.sync.dma_start(out=bq_sb, in_=bq_in[:, :])
            wk_sb = const.tile([128, KCH, CPC], DT)
            nc.sync.dma_start(out=wk_sb, in_=wkT.rearrange("(a p) c -> p a c", p=128))
            bk_sb = const.tile([CPC, 1], F32)
            nc.sync.dma_start(out=bk_sb, in_=bk_in[:, :])
            wv_sb = const.tile([128, KCH, CPC], DT)
            nc.sync.dma_start(out=wv_sb, in_=wvT.rearrange("(a p) c -> p a c", p=128))
            bv_sb = const.tile([CPC, 1], F32)
            nc.sync.dma_start(out=bv_sb, in_=bv_in[:, :])
            ident = const.tile([128, 128], DT)
            nc.sync.dma_start(out=ident, in_=ident_in[:, :])
            ones32 = const.tile([128, 32], F32)
            nc.vector.memset(ones32, 1.0)
            if causal:
                masks_sb = const.tile([128, 2, 4, QCH], DT)
                nc.sync.dma_start(out=masks_sb, in_=masks_in[:, :, :, :])

            qkT = {}
            vones = {}
            a2a_in = dram.tile([N_CORES * CPC, QCH], XDT)
            for b in range(B):
                qkT["q", b] = qk.tile([CPC, S], DT, name=f"qT{b}")
                qkT["k", b] = qk.tile([CPC, S], DT, name=f"kT{b}")
                vones[b] = qk.tile([128, NSB, HPC, HD + 1], DT, name=f"vones{b}")
                nc.vector.tensor_copy(vones[b][:, :, :, HD:HD + 1], ones32)

            def proj_half(b, sh):
                """Project q/k/v for s-range [1024*sh, 1024*(sh+1)) of batch b."""
                ssl = slice(1024 * sh, 1024 * (sh + 1))
                for tname, w_sb, bias_sb, xsrc in (
                    ("q", wq_sb, bq_sb, xqT),
                    ("k", wk_sb, bk_sb, xkT),
                    ("v", wv_sb, bv_sb, xvT),
                ):
                    pp = ps2.tile([128, 1024], F32, tag="ps2", name=f"pp{tname}{b}{sh}")
                    for a in range(KCH):
                        xt = xs.tile([128, 1024], DT, tag="xs", name=f"xt{tname}{b}{sh}{a}")
                        nc.sync.dma_start(out=xt, in_=xsrc[b, 128 * a:128 * (a + 1), ssl])
                        for m in range(2):
                            nc.tensor.matmul(pp[:, QCH * m:QCH * (m + 1)],
                                             lhsT=w_sb[:, a, :],
                                             rhs=xt[:, QCH * m:QCH * (m + 1)],
                                             start=(a == 0), stop=(a == KCH - 1))
                    if tname == "v":
                        vT = vtp.tile([CPC, 1024], DT, tag="vT", name=f"vT{b}{sh}")
                        nc.vector.tensor_scalar_add(vT, pp, bias_sb)
                        for i8 in range(0, 8, 2):
                            i = 8 * sh + i8
                            vp = ps2.tile([128, 2, 128], F32, tag="ps2", name=f"vp{b}{i}")
                            for u in range(2):
                                nc.tensor.matmul(
                                    vp[:, u, :],
                                    lhsT=vT[:, 128 * (i8 + u):128 * (i8 + u + 1)],
                                    rhs=ident, start=True, stop=True)
                            nc.vector.tensor_copy(
                                vones[b][:, i:i + 2, :, 0:HD],
                                vp.rearrange("p u (h d) -> p u h d", h=HPC))
                    else:
                        dst = qkT[tname, b]
                        nc.vector.tensor_scalar_add(
                            dst[:, 1024 * sh:1024 * (sh + 1)], pp, bias_sb)

            def attn_chunk(b, j):
                """Attention for q-chunk j of batch b; writes its a2a_in shard."""
                nblk = 4 * j + 4 if causal else NSB
                otp2 = ps2.tile([HD + 1, 2 * QCH], F32, tag="ps2", name=f"otp2{b}{j}")
                for i in range(nblk):
                    if not causal:
                        mt = xs.tile([128, QCH], F32, tag="mt", name=f"mt{b}{j}{i}")
                        nc.sync.dma_start(out=mt, in_=maskfull_in[i, :, j, :])
                    # causal straddle block: columns < 128*t are fully masked
                    t = i - 4 * j
                    c0 = 128 * t if (causal and t > 0) else 0
                    stp2 = ps2.tile([128, 2 * QCH], F32, tag="ps2", name=f"stp2{b}{j}{i}")
                    for h in range(HPC):
                        nc.tensor.matmul(
                            stp2[:, QCH * h + c0:QCH * (h + 1)],
                            lhsT=qkT["k", b][HD * h:HD * (h + 1), 128 * i:128 * (i + 1)],
                            rhs=qkT["q", b][HD * h:HD * (h + 1),
                                            QCH * j + c0:QCH * (j + 1)],
                            start=True, stop=True)
                    pt2 = pts.tile([128, 2, QCH], DT, tag="pt", name=f"pt2{b}{j}{i}")
                    sv = stp2.rearrange("p (h q) -> p h q", h=HPC)
                    nc.scalar.activation(out=pt2[:, :, c0:], in_=sv[:, :, c0:],
                                         func=AF.Exp, scale=SCALE)
                    if causal:
                        if t >= 0:
                            nc.vector.tensor_mul(pt2[:, :, c0:], pt2[:, :, c0:],
                                                 masks_sb[:, :, t, c0:])
                    else:
                        for h in range(HPC):
                            nc.vector.tensor_mul(pt2[:, h, :], pt2[:, h, :], mt)
                    for h in range(HPC):
                        nc.tensor.matmul(otp2[:, QCH * h + c0:QCH * (h + 1)],
                                         lhsT=vones[b][:, i, h, :],
                                         rhs=pt2[:, h, c0:],
                                         start=(i == 0), stop=(i == nblk - 1))
                for h in range(HPC):
                    rc = sm.tile([1, QCH], F32, tag="rc", name=f"rc{b}{j}{h}")
                    nc.vector.reciprocal(rc, otp2[HD:HD + 1, QCH * h:QCH * (h + 1)])
                    rb = sm.tile([HD, QCH], F32, tag="rb", name=f"rb{b}{j}{h}")
                    nc.gpsimd.partition_broadcast(rb, rc)
                    otn = sm.tile([HD, QCH], XDT, tag="otn", name=f"otn{b}{j}{h}")
                    nc.vector.tensor_mul(otn, otp2[0:HD, QCH * h:QCH * (h + 1)], rb)
                    base = CPC * (NQ * b + j) + HD * h
                    nc.sync.dma_start(out=a2a_in[base:base + HD, :], in_=otn)

            # ---- ladder: proj half / attention chunks ----
            proj_half(0, 0)
            proj_half(0, 1)
            attn_chunk(0, 0)
            attn_chunk(0, 1)
            proj_half(1, 0)
            proj_half(1, 1)
            attn_chunk(0, 2)
            attn_chunk(0, 3)
            attn_chunk(1, 0)
            attn_chunk(1, 1)
            attn_chunk(1, 2)
            attn_chunk(1, 3)

            # ---- late constants (needed from outproj onward) ----
            wo_sb = const.tile([128, KCH, D], XDT)
            nc.sync.dma_start(out=wo_sb, in_=woT.rearrange("(g p) d -> p g d", p=128))
            bo_row = const.tile([1, D], F32)
            nc.sync.dma_start(out=bo_row, in_=bo_in[:, :])
            bo_b = const.tile([128, D], F32)
            nc.gpsimd.partition_broadcast(bo_b, bo_row)
            gamma_row = const.tile([1, D], F32)
            nc.sync.dma_start(out=gamma_row, in_=gamma_in[:, :])
            gamma_b = const.tile([128, D], F32)
            nc.gpsimd.partition_broadcast(gamma_b, gamma_row)
            beta_row = const.tile([1, D], F32)
            nc.sync.dma_start(out=beta_row, in_=beta_in[:, :])
            beta_b = const.tile([128, D], F32)
            nc.gpsimd.partition_broadcast(beta_b, beta_row)
            eps_sb = const.tile([128, 1], F32)
            nc.vector.memset(eps_sb, EPS)

            # ---- exchange ----
            a2a_out = dram.tile([N_CORES * CPC, QCH], XDT)
            nc.gpsimd.collective_compute(
                "AllToAll", ALU.bypass,
                replica_groups=[list(range(N_CORES))],
                ins=[a2a_in[:]], outs=[a2a_out[:]],
            )
            agt = qk.tile([128, N_CORES, QCH], XDT, name="agt")
            nc.sync.dma_start(out=agt, in_=a2a_out.rearrange("(g p) m -> p g m", p=128))

            # ---- output projection + bias + residual + LayerNorm ----
            # residual + output bias preloaded and pre-added during attention
            rts = []
            for ms in range(4):
                rt = epi.tile([128, D], F32, tag="rt", name=f"rt{ms}", bufs=4)
                rts.append(rt)
                nc.sync.dma_start(
                    out=rt, in_=res_in.rearrange("(a p) d -> p a d", p=128)[:, ms, :])
                nc.vector.tensor_add(rt, rt, bo_b)
            for ms in range(4):
                pp2 = ps2.tile([128, D], F32, tag="ps2", name=f"op{ms}")
                for dh in range(2):
                    for g in range(N_CORES):
                        nc.tensor.matmul(pp2[:, 512 * dh:512 * (dh + 1)],
                                         lhsT=agt[:, g, 128 * ms:128 * (ms + 1)],
                                         rhs=wo_sb[:, g, 512 * dh:512 * (dh + 1)],
                                         start=(g == 0), stop=(g == N_CORES - 1))
                xt = epi.tile([128, D], F32, tag="xe", name=f"xe{ms}")
                nc.vector.tensor_add(xt, pp2, rts[ms])
                stats = sm.tile([128, 2, 6], F32, tag="stats", name=f"st{ms}")
                for g2 in range(2):
                    nc.vector.bn_stats(stats[:, g2, :], xt[:, 512 * g2:512 * (g2 + 1)])
                mv = sm.tile([128, 2], F32, tag="mv", name=f"mv{ms}")
                nc.vector.bn_aggr(mv, stats)
                sq = sm.tile([128, 1], F32, tag="sq", name=f"sq{ms}")
                nc.scalar.activation(out=sq, in_=mv[:, 1:2], func=AF.Sqrt, bias=eps_sb)
                rstd = sm.tile([128, 1], F32, tag="rstd", name=f"rs{ms}")
                nc.vector.reciprocal(rstd, sq)
                nc.vector.tensor_scalar(xt, xt, mv[:, 0:1], rstd,
                                        op0=ALU.subtract, op1=ALU.mult)
                nc.vector.tensor_mul(xt, xt, gamma_b)
                nc.vector.tensor_add(xt, xt, beta_b)
                nc.sync.dma_start(
                    out=out.rearrange("(a p) d -> p a d", p=128)[:, ms, :], in_=xt)

    nc.compile()
    return nc


def _get_program(causal: bool, dt_name: str, xdt_name: str):
    key = (causal, dt_name, xdt_name)
    if key not in _CACHE:
        _CACHE[key] = _build(causal, dt_name, xdt_name)
    return _CACHE[key]


def kernel(queries, keys, values, attention_ignore_mask,
           Wq, bq, Wk, bk, Wv, bv, Wo, bo, ln_gamma, ln_beta,
           dt_name="float32r", xdt_name="bfloat16"):
    from concourse.bass_utils import run_bass_kernel_spmd

    queries = np.asarray(queries, dtype=np.float32)
    keys = np.asarray(keys, dtype=np.float32)
    values = np.asarray(values, dtype=np.float32)
    mask = np.asarray(attention_ignore_mask)
    Wq, Wk, Wv, Wo = (np.asarray(w, dtype=np.float32) for w in (Wq, Wk, Wv, Wo))
    bq, bk, bv, bo = (np.asarray(x, dtype=np.float32) for x in (bq, bk, bv, bo))
    ln_gamma = np.asarray(ln_gamma, dtype=np.float32)
    ln_beta = np.asarray(ln_beta, dtype=np.float32)

    m2 = mask.reshape(S, S)
    causal = bool(np.array_equal(m2, np.triu(np.ones((S, S), m2.dtype), k=1)))

    import ml_dtypes
    np_dt = ml_dtypes.bfloat16 if dt_name == "bfloat16" else np.float32
    np_xdt = ml_dtypes.bfloat16 if xdt_name == "bfloat16" else np.float32

    nc = _get_program(causal, dt_name, xdt_name)

    xT = {}
    for nm, x in (("xqT", queries), ("xkT", keys), ("xvT", values)):
        t = np.empty((B, D, S), dtype=np_dt)
        for b in range(B):
            t[b] = x[b].T
        xT[nm] = t
    woT_np = np.ascontiguousarray(Wo.T, dtype=np_xdt)
    ident_np = np.ascontiguousarray(np.eye(128, dtype=np.float32), dtype=np_dt)
    bo_np = np.ascontiguousarray(bo.reshape(1, D))
    gamma_np = np.ascontiguousarray(ln_gamma.reshape(1, D))
    beta_np = np.ascontiguousarray(ln_beta.reshape(1, D))
    if causal:
        p = np.arange(128)[:, None]
        f = np.arange(QCH)[None, :]
        m1 = np.stack([(f >= p + 128 * t).astype(np.float32) for t in range(4)],
                      axis=1)  # [128, 4, 512]
        masks_np = np.ascontiguousarray(
            np.broadcast_to(m1[:, None], (128, HPC, 4, QCH)), dtype=np_dt)
    else:
        mm = np.ascontiguousarray(
            (m2 == 0).astype(np.float32).reshape(NSB, 128, NQ, QCH))

    in_maps = []
    for c in range(N_CORES):
        hsl = slice(CPC * c, CPC * (c + 1))
        bc, jc = c // NQ, c % NQ
        im = {
            "xqT": xT["xqT"], "xkT": xT["xkT"], "xvT": xT["xvT"],
            "wqT": np.ascontiguousarray(Wq[hsl, :].T, dtype=np_dt),
            "wkT": np.ascontiguousarray(Wk[hsl, :].T, dtype=np_dt),
            "wvT": np.ascontiguousarray(Wv[hsl, :].T, dtype=np_dt),
            "woT": woT_np, "ident_in": ident_np,
            "bq_in": np.ascontiguousarray(bq[hsl].reshape(CPC, 1)),
            "bk_in": np.ascontiguousarray(bk[hsl].reshape(CPC, 1)),
            "bv_in": np.ascontiguousarray(bv[hsl].reshape(CPC, 1)),
            "bo_in": bo_np, "gamma_in": gamma_np, "beta_in": beta_np,
            "res_in": np.ascontiguousarray(values[bc, QCH * jc:QCH * (jc + 1), :]),
        }
        if causal:
            im["masks_in"] = masks_np
        else:
            im["maskfull_in"] = mm
        in_maps.append(im)

    res = run_bass_kernel_spmd(nc, in_maps, core_ids=list(range(N_CORES)))
    out_full = np.empty((B, S, D), dtype=np.float32)
    for c in range(N_CORES):
        bc, jc = c // NQ, c % NQ
        out_full[bc, QCH * jc:QCH * (jc + 1), :] = res.results[c]["out"]
    return out_full


# revision 17
# speedup vs baseline: 1.1519x; 1.1519x over previous
"""Trainium2 Bass kernel for nn_MultiHeadMaskedAttention (B=2, S=2048, D=1024, H=16).

Sharding: 8-way tensor-parallel over heads (2 heads/core, both batches on every
core). Each core projects Q^T/K^T/V for its heads, runs causal attention in
transposed-score layout (S^T blocks = K^T-band.T @ Q^T-band, exp without
max-subtraction, denominator via a ones-column appended to V), normalizes, then
one 8-core AllToAll routes o^T columns so core c ends up with the full
1024-channel o^T for rows [512c, 512c+512) of concat(batch0, batch1). Each core
then does the output projection for its rows, adds bias + residual (values),
LayerNorms, and returns its [512, 1024] slice.

Matmul dtype is float32r (tf32-class, full PE rate at N=512) by default; the
exchange + output projection stage runs in bf16. Emission order ladders
projection s-halves with attention q-chunks so DMA streams under PE compute.
"""

import numpy as np

import concourse.bass as bass
import concourse.mybir as mybir
import concourse.tile as tile
from concourse import bacc

B, S, D, H = 2, 2048, 1024, 16
HD = D // H            # 64
N_CORES = 8
HPC = H // N_CORES     # 2 heads per core
CPC = HPC * HD         # 128 channels per core
SCALE = HD ** -0.5
EPS = 1e-12
QCH = 512              # q-chunk (free dim of S^T blocks)
NQ = S // QCH          # 4
NSB = S // 128         # 16 s-chunks
KCH = D // 128         # 8 contraction chunks

F32 = mybir.dt.float32
AF = mybir.ActivationFunctionType
ALU = mybir.AluOpType

_CACHE = {}


def _build(causal: bool, dt_name: str, xdt_name: str):
    DT = getattr(mybir.dt, dt_name)
    XDT = getattr(mybir.dt, xdt_name)
    nc = bacc.Bacc("TRN2", target_bir_lowering=False, debug=False,
                   num_devices=N_CORES)

    # ---- I/O ----
    xqT = nc.dram_tensor("xqT", [B, D, S], DT, kind="ExternalInput")
    xkT = nc.dram_tensor("xkT", [B, D, S], DT, kind="ExternalInput")
    xvT = nc.dram_tensor("xvT", [B, D, S], DT, kind="ExternalInput")
    wqT = nc.dram_tensor("wqT", [D, CPC], DT, kind="ExternalInput")
    wkT = nc.dram_tensor("wkT", [D, CPC], DT, kind="ExternalInput")
    wvT = nc.dram_tensor("wvT", [D, CPC], DT, kind="ExternalInput")
    woT = nc.dram_tensor("woT", [D, D], XDT, kind="ExternalInput")
    ident_in = nc.dram_tensor("ident_in", [128, 128], DT, kind="ExternalInput")
    bq_in = nc.dram_tensor("bq_in", [CPC, 1], F32, kind="ExternalInput")
    bk_in = nc.dram_tensor("bk_in", [CPC, 1], F32, kind="ExternalInput")
    bv_in = nc.dram_tensor("bv_in", [CPC, 1], F32, kind="ExternalInput")
    bo_in = nc.dram_tensor("bo_in", [1, D], F32, kind="ExternalInput")
    gamma_in = nc.dram_tensor("gamma_in", [1, D], F32, kind="ExternalInput")
    beta_in = nc.dram_tensor("beta_in", [1, D], F32, kind="ExternalInput")
    res_in = nc.dram_tensor("res_in", [QCH, D], F32, kind="ExternalInput")
    if causal:
        masks_in = nc.dram_tensor("masks_in", [128, 2, 4, QCH], DT, kind="ExternalInput")
    else:
        maskfull_in = nc.dram_tensor("maskfull_in", [NSB, 128, NQ, QCH], F32,
                                     kind="ExternalInput")
    out = nc.dram_tensor("out", [QCH, D], F32, kind="ExternalOutput")

    with tile.TileContext(nc) as tc:
        with (
            tc.tile_pool(name="const", bufs=1) as const,
            tc.tile_pool(name="xs", bufs=4) as xs,
            tc.tile_pool(name="qk", bufs=1) as qk,
            tc.tile_pool(name="vt", bufs=2) as vtp,
            tc.tile_pool(name="pts", bufs=6) as pts,
            tc.tile_pool(name="sm", bufs=4) as sm,
            tc.tile_pool(name="epi", bufs=2) as epi,
            tc.tile_pool(name="ps", bufs=2, space="PSUM") as ps,
            tc.tile_pool(name="ps2", bufs=3, space="PSUM") as ps2,
            tc.tile_pool(name="dram", bufs=1, space="DRAM") as dram,
        ):
            # ---- early constants (gpsimd DMA queue; xt streams use sync) ----
            wq_sb = const.tile([128, KCH, CPC], DT)
            nc.sync.dma_start(out=wq_sb, in_=wqT.rearrange("(a p) c -> p a c", p=128))
            bq_sb = const.tile([CPC, 1], F32)
            nc.sync.dma_start(out=bq_sb, in_=bq_in[:, :])
            wk_sb = const.tile([128, KCH, CPC], DT)
            nc.sync.dma_start(out=wk_sb, in_=wkT.rearrange("(a p) c -> p a c", p=128))
            bk_sb = const.tile([CPC, 1], F32)
            nc.sync.dma_start(out=bk_sb, in_=bk_in[:, :])
            wv_sb = const.tile([128, KCH, CPC], DT)
            nc.sync.dma_start(out=wv_sb, in_=wvT.rearrange("(a p) c -> p a c", p=128))
            bv_sb = const.tile([CPC, 1], F32)
            nc.sync.dma_start(out=bv_sb, in_=bv_in[:, :])
            ident = const.tile([128, 128], DT)
            nc.sync.dma_start(out=ident, in_=ident_in[:, :])
            ones32 = const.tile([128, 32], F32)
            nc.vector.memset(ones32, 1.0)
            if causal:
                masks_sb = const.tile([128, 2, 4, QCH], DT)
                nc.sync.dma_start(out=masks_sb, in_=masks_in[:, :, :, :])

            qkT = {}
            vones = {}
            a2a_in = dram.tile([N_CORES * CPC, QCH], XDT)
            for b in range(B):
                qkT["q", b] = qk.tile([CPC, S], DT, name=f"qT{b}")
                qkT["k", b] = qk.tile([CPC, S], DT, name=f"kT{b}")
                vones[b] = qk.tile([128, NSB, HPC, HD + 1], DT, name=f"vones{b}")
                nc.vector.tensor_copy(vones[b][:, :, :, HD:HD + 1], ones32)

            def proj_half(b, sh):
                """Project q/k/v for s-range [1024*sh, 1024*(sh+1)) of batch b."""
                ssl = slice(1024 * sh, 1024 * (sh + 1))
                for tname, w_sb, bias_sb, xsrc in (
                    ("q", wq_sb, bq_sb, xqT),
                    ("k", wk_sb, bk_sb, xkT),
                    ("v", wv_sb, bv_sb, xvT),
                ):
                    pps = [ps.tile([128, QCH], F32, tag="ps", name=f"pp{tname}{b}{sh}{m}")
                           for m in range(2)]
                    for a in range(KCH):
                        xt = xs.tile([128, 1024], DT, tag="xs", name=f"xt{tname}{b}{sh}{a}")
                        nc.sync.dma_start(out=xt, in_=xsrc[b, 128 * a:128 * (a + 1), ssl])
                        for m in range(2):
                            nc.tensor.matmul(pps[m], lhsT=w_sb[:, a, :],
                                             rhs=xt[:, QCH * m:QCH * (m + 1)],
                                             start=(a == 0), stop=(a == KCH - 1))
                    if tname == "v":
                        vT = vtp.tile([CPC, 1024], DT, tag="vT", name=f"vT{b}{sh}")
                        for m in range(2):
                            nc.vector.tensor_scalar_add(
                                vT[:, QCH * m:QCH * (m + 1)], pps[m], bias_sb)
                        for i8 in range(0, 8, 2):
                            i = 8 * sh + i8
                            vp = ps.tile([128, 2, 128], F32, tag="ps", name=f"vp{b}{i}")
                            for u in range(2):
                                nc.tensor.matmul(
                                    vp[:, u, :],
                                    lhsT=vT[:, 128 * (i8 + u):128 * (i8 + u + 1)],
                                    rhs=ident, start=True, stop=True)
                            nc.vector.tensor_copy(
                                vones[b][:, i:i + 2, :, 0:HD],
                                vp.rearrange("p u (h d) -> p u h d", h=HPC))
                    else:
                        dst = qkT[tname, b]
                        for m in range(2):
                            nc.vector.tensor_scalar_add(
                                dst[:, 1024 * sh + QCH * m:1024 * sh + QCH * (m + 1)],
                                pps[m], bias_sb)

            def attn_chunk(b, j):
                """Attention for q-chunk j of batch b; writes its a2a_in shard."""
                nblk = 4 * j + 4 if causal else NSB
                otp2 = ps2.tile([HD + 1, 2 * QCH], F32, tag="ps2", name=f"otp2{b}{j}")
                for i in range(nblk):
                    if not causal:
                        mt = xs.tile([128, QCH], F32, tag="mt", name=f"mt{b}{j}{i}")
                        nc.sync.dma_start(out=mt, in_=maskfull_in[i, :, j, :])
                    # causal straddle block: columns < 128*t are fully masked
                    t = i - 4 * j
                    c0 = 128 * t if (causal and t > 0) else 0
                    stp2 = ps2.tile([128, 2 * QCH], F32, tag="ps2", name=f"stp2{b}{j}{i}")
                    for h in range(HPC):
                        nc.tensor.matmul(
                            stp2[:, QCH * h + c0:QCH * (h + 1)],
                            lhsT=qkT["k", b][HD * h:HD * (h + 1), 128 * i:128 * (i + 1)],
                            rhs=qkT["q", b][HD * h:HD * (h + 1),
                                            QCH * j + c0:QCH * (j + 1)],
                            start=True, stop=True)
                    pt2 = pts.tile([128, 2, QCH], DT, tag="pt", name=f"pt2{b}{j}{i}")
                    sv = stp2.rearrange("p (h q) -> p h q", h=HPC)
                    nc.scalar.activation(out=pt2[:, :, c0:], in_=sv[:, :, c0:],
                                         func=AF.Exp, scale=SCALE)
                    if causal:
                        if t >= 0:
                            nc.vector.tensor_mul(pt2[:, :, c0:], pt2[:, :, c0:],
                                                 masks_sb[:, :, t, c0:])
                    else:
                        for h in range(HPC):
                            nc.vector.tensor_mul(pt2[:, h, :], pt2[:, h, :], mt)
                    for h in range(HPC):
                        nc.tensor.matmul(otp2[:, QCH * h + c0:QCH * (h + 1)],
                                         lhsT=vones[b][:, i, h, :],
                                         rhs=pt2[:, h, c0:],
                                         start=(i == 0), stop=(i == nblk - 1))
                for h in range(HPC):
                    rc = sm.tile([1, QCH], F32, tag="rc", name=f"rc{b}{j}{h}")
                    nc.vector.reciprocal(rc, otp2[HD:HD + 1, QCH * h:QCH * (h + 1)])
                    rb = sm.tile([HD, QCH], F32, tag="rb", name=f"rb{b}{j}{h}")
                    nc.gpsimd.partition_broadcast(rb, rc)
                    otn = sm.tile([HD, QCH], XDT, tag="otn", name=f"otn{b}{j}{h}")
                    nc.vector.tensor_mul(otn, otp2[0:HD, QCH * h:QCH * (h + 1)], rb)
                    base = CPC * (NQ * b + j) + HD * h
                    nc.sync.dma_start(out=a2a_in[base:base + HD, :], in_=otn)

            # ---- ladder: proj half / attention chunks ----
            proj_half(0, 0)
            proj_half(0, 1)
            attn_chunk(0, 0)
            attn_chunk(0, 1)
            proj_half(1, 0)
            proj_half(1, 1)
            attn_chunk(0, 2)
            attn_chunk(0, 3)
            attn_chunk(1, 0)
            attn_chunk(1, 1)
            attn_chunk(1, 2)
            attn_chunk(1, 3)

            # ---- late constants (needed from outproj onward) ----
            wo_sb = const.tile([128, KCH, D], XDT)
            nc.sync.dma_start(out=wo_sb, in_=woT.rearrange("(g p) d -> p g d", p=128))
            bo_row = const.tile([1, D], F32)
            nc.sync.dma_start(out=bo_row, in_=bo_in[:, :])
            bo_b = const.tile([128, D], F32)
            nc.gpsimd.partition_broadcast(bo_b, bo_row)
            gamma_row = const.tile([1, D], F32)
            nc.sync.dma_start(out=gamma_row, in_=gamma_in[:, :])
            gamma_b = const.tile([128, D], F32)
            nc.gpsimd.partition_broadcast(gamma_b, gamma_row)
            beta_row = const.tile([1, D], F32)
            nc.sync.dma_start(out=beta_row, in_=beta_in[:, :])
            beta_b = const.tile([128, D], F32)
            nc.gpsimd.partition_broadcast(beta_b, beta_row)
            eps_sb = const.tile([128, 1], F32)
            nc.vector.memset(eps_sb, EPS)

            # ---- exchange ----
            a2a_out = dram.tile([N_CORES * CPC, QCH], XDT)
            nc.gpsimd.collective_compute(
                "AllToAll", ALU.bypass,
                replica_groups=[list(range(N_CORES))],
                ins=[a2a_in[:]], outs=[a2a_out[:]],
            )
            agt = qk.tile([128, N_CORES, QCH], XDT, name="agt")
            nc.sync.dma_start(out=agt, in_=a2a_out.rearrange("(g p) m -> p g m", p=128))

            # ---- output projection + bias + residual + LayerNorm ----
            # residual + output bias preloaded and pre-added during attention
            rts = []
            for ms in range(4):
                rt = epi.tile([128, D], F32, tag="rt", name=f"rt{ms}", bufs=4)
                rts.append(rt)
                nc.sync.dma_start(
                    out=rt, in_=res_in.rearrange("(a p) d -> p a d", p=128)[:, ms, :])
                nc.vector.tensor_add(rt, rt, bo_b)
            for ms in range(4):
                pp2 = ps2.tile([128, D], F32, tag="ps2", name=f"op{ms}")
                for dh in range(2):
                    for g in range(N_CORES):
                        nc.tensor.matmul(pp2[:, 512 * dh:512 * (dh + 1)],
                                         lhsT=agt[:, g, 128 * ms:128 * (ms + 1)],
                                         rhs=wo_sb[:, g, 512 * dh:512 * (dh + 1)],
                                         start=(g == 0), stop=(g == N_CORES - 1))
                xt = epi.tile([128, D], F32, tag="xe", name=f"xe{ms}")
                nc.vector.tensor_add(xt, pp2, rts[ms])
                stats = sm.tile([128, 2, 6], F32, tag="stats", name=f"st{ms}")
                for g2 in range(2):
                    nc.vector.bn_stats(stats[:, g2, :], xt[:, 512 * g2:512 * (g2 + 1)])
                mv = sm.tile([128, 2], F32, tag="mv", name=f"mv{ms}")
                nc.vector.bn_aggr(mv, stats)
                sq = sm.tile([128, 1], F32, tag="sq", name=f"sq{ms}")
                nc.scalar.activation(out=sq, in_=mv[:, 1:2], func=AF.Sqrt, bias=eps_sb)
                rstd = sm.tile([128, 1], F32, tag="rstd", name=f"rs{ms}")
                nc.vector.reciprocal(rstd, sq)
                nc.vector.tensor_scalar(xt, xt, mv[:, 0:1], rstd,
                                        op0=ALU.subtract, op1=ALU.mult)
                nc.vector.tensor_mul(xt, xt, gamma_b)
                nc.vector.tensor_add(xt, xt, beta_b)
                nc.sync.dma_start(
                    out=out.rearrange("(a p) d -> p a d", p=128)[:, ms, :], in_=xt)

    nc.compile()
    return nc


def _get_program(causal: bool, dt_name: str, xdt_name: str):
    key = (causal, dt_name, xdt_name)
    if key not in _CACHE:
        _CACHE[key] = _build(causal, dt_name, xdt_name)
    return _CACHE[key]


def kernel(queries, keys, values, attention_ignore_mask,
           Wq, bq, Wk, bk, Wv, bv, Wo, bo, ln_gamma, ln_beta,
           dt_name="float32r", xdt_name="bfloat16"):
    from concourse.bass_utils import run_bass_kernel_spmd

    queries = np.asarray(queries, dtype=np.float32)
    keys = np.asarray(keys, dtype=np.float32)
    values = np.asarray(values, dtype=np.float32)
    mask = np.asarray(attention_ignore_mask)
    Wq, Wk, Wv, Wo = (np.asarray(w, dtype=np.float32) for w in (Wq, Wk, Wv, Wo))
    bq, bk, bv, bo = (np.asarray(x, dtype=np.float32) for x in (bq, bk, bv, bo))
    ln_gamma = np.asarray(ln_gamma, dtype=np.float32)
    ln_beta = np.asarray(ln_beta, dtype=np.float32)

    m2 = mask.reshape(S, S)
    causal = bool(np.array_equal(m2, np.triu(np.ones((S, S), m2.dtype), k=1)))

    import ml_dtypes
    np_dt = ml_dtypes.bfloat16 if dt_name == "bfloat16" else np.float32
    np_xdt = ml_dtypes.bfloat16 if xdt_name == "bfloat16" else np.float32

    nc = _get_program(causal, dt_name, xdt_name)

    xT = {}
    for nm, x in (("xqT", queries), ("xkT", keys), ("xvT", values)):
        t = np.empty((B, D, S), dtype=np_dt)
        for b in range(B):
            t[b] = x[b].T
        xT[nm] = t
    woT_np = np.ascontiguousarray(Wo.T, dtype=np_xdt)
    ident_np = np.ascontiguousarray(np.eye(128, dtype=np.float32), dtype=np_dt)
    bo_np = np.ascontiguousarray(bo.reshape(1, D))
    gamma_np = np.ascontiguousarray(ln_gamma.reshape(1, D))
    beta_np = np.ascontiguousarray(ln_beta.reshape(1, D))
    if causal:
        p = np.arange(128)[:, None]
        f = np.arange(QCH)[None, :]
        m1 = np.stack([(f >= p + 128 * t).astype(np.float32) for t in range(4)],
                      axis=1)  # [128, 4, 512]
        masks_np = np.ascontiguousarray(
            np.broadcast_to(m1[:, None], (128, HPC, 4, QCH)), dtype=np_dt)
    else:
        mm = np.ascontiguousarray(
            (m2 == 0).astype(np.float32).reshape(NSB, 128, NQ, QCH))

    in_maps = []
    for c in range(N_CORES):
        hsl = slice(CPC * c, CPC * (c + 1))
        bc, jc = c // NQ, c % NQ
        im = {
            "xqT": xT["xqT"], "xkT": xT["xkT"], "xvT": xT["xvT"],
            "wqT": np.ascontiguousarray(Wq[hsl, :].T, dtype=np_dt),
            "wkT": np.ascontiguousarray(Wk[hsl, :].T, dtype=np_dt),
            "wvT": np.ascontiguousarray(Wv[hsl, :].T, dtype=np_dt),
            "woT": woT_np, "ident_in": ident_np,
            "bq_in": np.ascontiguousarray(bq[hsl].reshape(CPC, 1)),
            "bk_in": np.ascontiguousarray(bk[hsl].reshape(CPC, 1)),
            "bv_in": np.ascontiguousarray(bv[hsl].reshape(CPC, 1)),
            "bo_in": bo_np, "gamma_in": gamma_np, "beta_in": beta_np,
            "res_in": np.ascontiguousarray(values[bc, QCH * jc:QCH * (jc + 1), :]),
        }
        if causal:
            im["masks_in"] = masks_np
        else:
            im["maskfull_in"] = mm
        in_maps.append(im)

    res = run_bass_kernel_spmd(nc, in_maps, core_ids=list(range(N_CORES)))
    out_full = np.empty((B, S, D), dtype=np.float32)
    for c in range(N_CORES):
        bc, jc = c // NQ, c % NQ
        out_full[bc, QCH * jc:QCH * (jc + 1), :] = res.results[c]["out"]
    return out_full
